# revision 1
# baseline (speedup 1.0000x reference)
"""GCN (3-layer GCNConv + mean-pool + MLP head) Trainium2 Bass kernel, 8 NeuronCores.

Strategy (graph/data parallel, per sharding hint):
  - Destination nodes are partitioned into 8 contiguous blocks (one per core).
  - Host partitions the edge list (with self-loops) by destination block, sorts
    by destination window (128 dst nodes per window), and lays edges out on a
    [128, NW*T] grid so each 128-edge tile feeds one PE matmul.
  - Per layer, each core gathers source-node features for its edges via
    indirect DMA, multiplies by the GCN edge norm, and segment-sums into its
    destination windows with one-hot x message matmuls accumulated in PSUM
    (aggregate-then-transform: A_hat @ (h W) == (A_hat @ h) @ W, so gathers run
    at the *input* feature width).
  - The per-shard layer output (relu(agg @ W + b)) is written to a shard
    bounce buffer and AllGathered so every core has the full node-feature
    table for the next layer's gathers.
  - Layer 3 output is mean-pooled per graph locally (one-hot matmul into a
    PSUM accumulator held across the layer), AllReduced across cores, and the
    tiny FC head runs replicated on every core.
"""

import os
import sys
from dataclasses import dataclass, field

import numpy as np
import ml_dtypes

for _p in ("/opt/trn_rl_repo", "/root/.axon_site/_ro/trn_rl_repo"):
    if os.path.isdir(_p) and _p not in sys.path:
        sys.path.insert(0, _p)

bf16 = ml_dtypes.bfloat16
P = 128


@dataclass
class GCNConfig:
    N: int = 100000          # real nodes
    G: int = 128             # graphs (output width; PSUM col budget)
    SHARD: int = 12544       # padded nodes per core (NW * 128)
    NW: int = 98             # dst windows per core
    T: int = 17              # edge columns per window (self-loops excluded; auto-derived per input)
    CHUNK_W: int = 7         # windows per gather chunk
    F: tuple = (40, 40, 80, 160)   # feature dims x -> h1 -> h2 -> h3
    HID: int = 128           # fc hidden
    n_cores: int = 8

    @property
    def NPAD(self):
        return self.n_cores * self.SHARD

    @property
    def COLS(self):
        return self.NW * self.T


CFG = GCNConfig()


# ---------------------------------------------------------------- host prep

def build_host_data(cfg, inp):
    """Partition/sort edges by destination block, compute GCN edge norms,
    build per-core edge grids and tables. All index/structure metadata."""
    N, SHARD, NW, T = cfg.N, cfg.SHARD, cfg.NW, cfg.T
    src = np.asarray(inp["edge_index"][0]).astype(np.int64).ravel()
    dst = np.asarray(inp["edge_index"][1]).astype(np.int64).ravel()
    batch = np.asarray(inp["batch"]).astype(np.int64).ravel()
    # degree includes self-loops (as in GCNConv); self-loops themselves are
    # handled by a dense per-window diagonal term, not by the edge grid
    deg = (np.bincount(dst, minlength=N) + 1).astype(np.float32)
    dis = 1.0 / np.sqrt(deg)
    norm = (dis[src] * dis[dst]).astype(np.float32)
    srcA, dstA = src, dst

    core = dstA // SHARD
    win = (dstA % SHARD) // P
    dloc = (dstA % SHARD) % P
    order = np.lexsort((win, core))
    srcA, core, win, dloc, norm = (a[order] for a in (srcA, core, win, dloc, norm))

    cores = []
    for c in range(cfg.n_cores):
        m = core == c
        sc, dl, nm, wn = srcA[m], dloc[m], norm[m], win[m]
        esrc = np.zeros((P, NW * T), np.int32)
        edl = np.full((P, NW * T), -1.0, bf16)
        enrm = np.zeros((P, NW * T), bf16)
        # edges of window w occupy grid slots [p, w*T + j//128]
        wcnt = np.bincount(wn, minlength=NW)
        assert wcnt.max() <= T * P, f"window overflow: {wcnt.max()} > {T * P}"
        jin = np.concatenate([np.arange(n) for n in wcnt]) if len(wn) else np.zeros(0, np.int64)
        pp = jin % P
        cc = wn * T + jin // P
        esrc[pp, cc] = sc
        edl[pp, cc] = dl.astype(bf16)
        enrm[pp, cc] = nm.astype(bf16)
        nid = np.arange(SHARD) + c * SHARD
        gl = np.where(nid < N, batch[np.minimum(nid, N - 1)], -1).astype(np.float32)
        gloc = np.ascontiguousarray(gl.reshape(NW, P).T).astype(bf16)
        d2 = np.where(nid < N, 1.0 / deg[np.minimum(nid, N - 1)], 0.0).astype(np.float32)
        dis2 = np.ascontiguousarray(d2.reshape(NW, P).T).astype(bf16)
        xs = np.zeros((SHARD, cfg.F[0]), bf16)
        nreal = max(0, min(SHARD, N - c * SHARD))
        xs[:nreal] = np.asarray(inp["x"])[c * SHARD:c * SHARD + nreal].astype(bf16)
        cores.append(dict(esrc=esrc, edl=edl, enrm=enrm, gloc=gloc, dis2=dis2, xs=xs))

    # segment-major table layout: two AllGather segments per layer boundary.
    # node v -> row seg*(n_cores*SEG) + core*SEG + (v%SHARD - seg*SEG),
    # with SEG = SHARD//2; segment s of every core is one collective.
    SEG = SHARD // 2
    vv = np.arange(cfg.NPAD, dtype=np.int64)
    vc, vr = vv // SHARD, vv % SHARD
    vs = vr // SEG
    remap = vs * (cfg.n_cores * SEG) + vc * SEG + (vr - vs * SEG)
    for d in cores:
        d["esrc"] = remap[d["esrc"].astype(np.int64)].astype(np.int32)
    xt = np.zeros((cfg.NPAD, cfg.F[0]), bf16)
    xr = np.asarray(inp["x"]).astype(bf16)
    xt[remap[:N]] = xr

    cnt = np.bincount(batch, minlength=cfg.G).astype(np.float32)
    invc = np.zeros((P, 1), np.float32)
    invc[:cfg.G, 0] = 1.0 / np.maximum(cnt, 1.0)

    def a2(x, dt):
        return np.ascontiguousarray(np.asarray(x), dtype=dt)

    wts = dict(
        w1a=np.concatenate([a2(inp["W1"], bf16), a2(inp["b1"], bf16)[None]], 0),
        w2a=np.concatenate([a2(inp["W2"], bf16), a2(inp["b2"], bf16)[None]], 0),
        w3a=np.concatenate([a2(inp["W3"], bf16), a2(inp["b3"], bf16)[None]], 0),
        fw1=a2(inp["fW1"], bf16),
        fb1c=a2(inp["fb1"], np.float32).reshape(-1, 1),
        fw2=a2(inp["fW2"], bf16),
        invc=invc,
    )
    fb2 = float(np.asarray(inp["fb2"]).ravel()[0])
    return cores, xt, wts, fb2


# ---------------------------------------------------------------- bass build

def build_bass(cfg, fb2):
    import concourse.bacc as bacc
    import concourse.bass as bass
    import concourse.mybir as mybir
    import concourse.tile as tile
    from concourse.masks import make_identity

    dt = mybir.dt
    AF = mybir.ActivationFunctionType
    OP = mybir.AluOpType
    F0, F1, F2, F3 = cfg.F
    NW, T, CW, G = cfg.NW, cfg.T, cfg.CHUNK_W, cfg.G
    NCH = NW // CW
    assert NW % CW == 0
    FMAX = max(F0, F1, F2)

    nc = bacc.Bacc("TRN2", target_bir_lowering=False, debug=False,
                   enable_asserts=False, num_devices=cfg.n_cores,
                   num_swdge_queues=4)

    # ---- I/O
    xt_d = nc.dram_tensor("xt", [cfg.NPAD, F0], dt.bfloat16, kind="ExternalInput")
    esrc_d = nc.dram_tensor("esrc", [P, cfg.COLS], dt.int32, kind="ExternalInput")
    edl_d = nc.dram_tensor("edl", [P, cfg.COLS], dt.bfloat16, kind="ExternalInput")
    enrm_d = nc.dram_tensor("enrm", [P, cfg.COLS], dt.bfloat16, kind="ExternalInput")
    gloc_d = nc.dram_tensor("gloc", [P, NW], dt.bfloat16, kind="ExternalInput")
    dis2_d = nc.dram_tensor("dis2", [P, NW], dt.bfloat16, kind="ExternalInput")
    xs_d = nc.dram_tensor("xs", [cfg.SHARD, F0], dt.bfloat16, kind="ExternalInput")
    w1a_d = nc.dram_tensor("w1a", [F0 + 1, F1], dt.bfloat16, kind="ExternalInput")
    w2a_d = nc.dram_tensor("w2a", [F1 + 1, F2], dt.bfloat16, kind="ExternalInput")
    w3a_d = nc.dram_tensor("w3a", [F2 + 1, F3], dt.bfloat16, kind="ExternalInput")
    fw1_d = nc.dram_tensor("fw1", [F3, cfg.HID], dt.bfloat16, kind="ExternalInput")
    fb1_d = nc.dram_tensor("fb1c", [cfg.HID, 1], dt.float32, kind="ExternalInput")
    fw2_d = nc.dram_tensor("fw2", [cfg.HID, 1], dt.bfloat16, kind="ExternalInput")
    invc_d = nc.dram_tensor("invc", [P, 1], dt.float32, kind="ExternalInput")
    out_d = nc.dram_tensor("out", [1, P], dt.float32, kind="ExternalOutput")

    rg = [list(range(cfg.n_cores))]

    with tile.TileContext(nc) as tc:
        with (
            tc.tile_pool(name="res", bufs=1) as res,                  # persistent SBUF
            tc.tile_pool(name="msgs", bufs=3) as msgsp,
            tc.tile_pool(name="sp", bufs=3) as sp,
            tc.tile_pool(name="work", bufs=2) as work,
            tc.tile_pool(name="pa_ps", bufs=2, space="PSUM") as pa_ps,
            tc.tile_pool(name="p2_ps", bufs=2, space="PSUM") as p2_ps,
            tc.tile_pool(name="pool_ps", bufs=1, space="PSUM") as pool_ps,
            tc.tile_pool(name="head_ps", bufs=1, space="PSUM") as head_ps,
            tc.tile_pool(name="dram", bufs=1, space="DRAM") as dram,
        ):
            # ---- load persistent SBUF state
            esrc = res.tile([P, cfg.COLS], dt.int32)
            edl = res.tile([P, cfg.COLS], dt.bfloat16)
            enrm = res.tile([P, cfg.COLS], dt.bfloat16)
            gloc = res.tile([P, NW], dt.bfloat16)
            dis2 = res.tile([P, NW], dt.bfloat16)
            w1a = res.tile([F0 + 1, F1], dt.bfloat16)
            w2a = res.tile([F1 + 1, F2], dt.bfloat16)
            w3a = res.tile([F2 + 1, F3], dt.bfloat16)
            fw1a = res.tile([F3 // 2, cfg.HID], dt.bfloat16)
            fw1b = res.tile([F3 // 2, cfg.HID], dt.bfloat16)
            fb1c = res.tile([cfg.HID, 1], dt.float32)
            fw2 = res.tile([cfg.HID, 1], dt.bfloat16)
            invc = res.tile([P, 1], dt.float32)
            b1r = res.tile([1, F1], dt.bfloat16)
            b2r = res.tile([1, F2], dt.bfloat16)
            b3r = res.tile([1, F3], dt.bfloat16)
            for sb, dr in ((esrc, esrc_d), (edl, edl_d), (enrm, enrm_d),
                           (gloc, gloc_d), (dis2, dis2_d), (w1a, w1a_d), (w2a, w2a_d),
                           (w3a, w3a_d), (fb1c, fb1_d), (fw2, fw2_d),
                           (invc, invc_d)):
                nc.sync.dma_start(out=sb[:], in_=dr[:])
            nc.sync.dma_start(out=b1r[:], in_=w1a_d[F0:F0 + 1, :])
            nc.sync.dma_start(out=b2r[:], in_=w2a_d[F1:F1 + 1, :])
            nc.sync.dma_start(out=b3r[:], in_=w3a_d[F2:F2 + 1, :])
            nc.sync.dma_start(out=fw1a[:], in_=fw1_d[0:F3 // 2, :])
            nc.sync.dma_start(out=fw1b[:], in_=fw1_d[F3 // 2:, :])

            iota_i = res.tile([P, P], dt.int32)
            nc.gpsimd.iota(iota_i[:], pattern=[[1, P]], base=0, channel_multiplier=0)
            iota_b = res.tile([P, P], dt.bfloat16)
            nc.vector.tensor_copy(out=iota_b[:], in_=iota_i[:])
            ident = res.tile([P, P], dt.bfloat16)
            make_identity(nc, ident[:])
            ones1 = res.tile([1, P], dt.bfloat16)
            nc.vector.memset(ones1[:], 1.0)

            # ---- DRAM tables / bounce buffers
            h1s = dram.tile([cfg.SHARD, F1], dt.bfloat16)
            h2s = dram.tile([cfg.SHARD, F2], dt.bfloat16)
            h1t = dram.tile([cfg.NPAD, F1], dt.bfloat16)
            h2t = dram.tile([cfg.NPAD, F2], dt.bfloat16)
            pool_pt = dram.tile([P, F3], dt.float32)
            pool_rd = dram.tile([P, F3], dt.float32)

            pool_acc = pool_ps.tile([P, F3], dt.float32)

            def layer(tbl, F_in, F_out, waug, brow, self_src, shard_out):
                last = F_in == F2  # layer 3
                for w in range(NW):
                    wc = slice(w * T, (w + 1) * T)
                    msgs = msgsp.tile([P, T, FMAX], dt.bfloat16, tag="msgs", name="msgs")
                    msgs = msgs[:, :, :F_in]
                    for t in range(T):
                        gi = nc.gpsimd.indirect_dma_start(
                            out=msgs[:, t, :],
                            out_offset=None,
                            in_=tbl[:],
                            in_offset=bass.IndirectOffsetOnAxis(
                                ap=esrc[:, w * T + t:w * T + t + 1], axis=0),
                        )
                        q = t % 4
                        if q:
                            gi.queue = f"qPoolDynamic{q}"
                    if True:
                        S = sp.tile([P, T, P], dt.bfloat16, tag="S", name="S")
                        nc.vector.tensor_tensor(
                            out=S[:],
                            in0=edl[:, wc, None].broadcast_to([P, T, P]),
                            in1=iota_b[:, None, :].broadcast_to([P, T, P]),
                            op=OP.is_equal)
                        mw = msgs
                        nc.vector.tensor_tensor(
                            out=mw, in0=mw,
                            in1=enrm[:, wc, None].broadcast_to([P, T, F_in]),
                            op=OP.mult)
                        pa = pa_ps.tile([FMAX, P], dt.float32, tag="pa", name="pa")[:F_in]
                        for t in range(T):
                            nc.tensor.matmul(
                                out=pa[:], lhsT=mw[:, t, :], rhs=S[:, t, :],
                                start=(t == 0), stop=False)
                        # self-loop term: aggT[f,d] += h[d,f] * dis2[d] via
                        # matmul with a diagonal rhs (local shard rows, no gather)
                        hw_t = work.tile([P, FMAX], dt.bfloat16, tag="hw", name="hw_t")[:, :F_in]
                        nc.sync.dma_start(out=hw_t[:], in_=self_src[w * P:(w + 1) * P, :])
                        Dd = sp.tile([P, P], dt.bfloat16, tag="Dd", name="Dd")
                        nc.vector.tensor_tensor(
                            out=Dd[:], in0=ident[:],
                            in1=dis2[:, w:w + 1].broadcast_to([P, P]), op=OP.mult)
                        nc.tensor.matmul(out=pa[:], lhsT=hw_t[:], rhs=Dd[:],
                                         start=False, stop=True)
                        aggT = work.tile([FMAX, P], dt.bfloat16, tag="aggT", name="aggT")[:F_in]
                        nc.scalar.copy(out=aggT[:], in_=pa[:])
                        p2 = p2_ps.tile([P, F3], dt.float32, tag="p2", name="p2")[:, :F_out]
                        nc.tensor.matmul(out=p2[:], lhsT=aggT[:], rhs=waug[:F_in, :],
                                         start=True, stop=False)
                        nc.tensor.matmul(out=p2[:], lhsT=ones1[:], rhs=brow[:],
                                         start=False, stop=True)
                        h = work.tile([P, F3], dt.bfloat16, tag="h", name="h")[:, :F_out]
                        nc.scalar.activation(h[:], p2[:], AF.Relu)
                        if not last:
                            nc.sync.dma_start(out=shard_out[w * P:(w + 1) * P, :], in_=h[:])
                        else:
                            Sg = sp.tile([P, P], dt.bfloat16, tag="Sg", name="Sg")
                            nc.vector.tensor_tensor(
                                out=Sg[:],
                                in0=gloc[:, w:w + 1].broadcast_to([P, P]),
                                in1=iota_b[:], op=OP.is_equal)
                            nc.tensor.matmul(out=pool_acc[:], lhsT=Sg[:], rhs=h[:],
                                             start=(w == 0), stop=(w == NW - 1))

            SEG = cfg.SHARD // 2
            HSEG = cfg.n_cores * SEG

            def seg_allgather(shard, table):
                for sgi in range(2):
                    nc.gpsimd.collective_compute(
                        "AllGather", mybir.AluOpType.bypass, replica_groups=rg,
                        ins=[shard[sgi * SEG:(sgi + 1) * SEG, :].opt()],
                        outs=[table[sgi * HSEG:(sgi + 1) * HSEG, :].opt()])

            layer(xt_d, F0, F1, w1a, b1r, xs_d, h1s)
            seg_allgather(h1s, h1t)
            layer(h1t, F1, F2, w2a, b2r, h1s, h2s)
            seg_allgather(h2s, h2t)
            layer(h2t, F2, F3, w3a, b3r, h2s, None)

            # ---- pooling partial -> AllReduce -> mean
            psb = work.tile([P, F3], dt.float32, tag="psb")
            nc.scalar.copy(out=psb[:], in_=pool_acc[:])
            nc.sync.dma_start(out=pool_pt[:], in_=psb[:])
            nc.gpsimd.collective_compute(
                "AllReduce", mybir.AluOpType.add, replica_groups=rg,
                ins=[pool_pt.opt()], outs=[pool_rd.opt()])
            poolr = work.tile([P, F3], dt.float32, tag="poolr")
            nc.sync.dma_start(out=poolr[:], in_=pool_rd[:])
            pooled = work.tile([P, F3], dt.bfloat16, tag="pooled")
            nc.scalar.activation(pooled[:], poolr[:], AF.Copy, scale=invc[:])

            # ---- head: z1 = relu(pooled @ fW1 + fb1); z2 = z1 @ fW2 + fb2
            ptA_ps = head_ps.tile([F3 // 2, P], dt.bfloat16, tag="pt")
            nc.tensor.transpose(out=ptA_ps[:], in_=pooled[:, :F3 // 2], identity=ident[:])
            ptA = work.tile([F3 // 2, P], dt.bfloat16, tag="ptA")
            nc.scalar.copy(out=ptA[:], in_=ptA_ps[:])
            ptB_ps = head_ps.tile([F3 // 2, P], dt.bfloat16, tag="pt")
            nc.tensor.transpose(out=ptB_ps[:], in_=pooled[:, F3 // 2:], identity=ident[:])
            ptB = work.tile([F3 // 2, P], dt.bfloat16, tag="ptB")
            nc.scalar.copy(out=ptB[:], in_=ptB_ps[:])

            z1_ps = head_ps.tile([cfg.HID, P], dt.float32, tag="z1")
            nc.tensor.matmul(out=z1_ps[:], lhsT=fw1a[:], rhs=ptA[:], start=True, stop=False)
            nc.tensor.matmul(out=z1_ps[:], lhsT=fw1b[:], rhs=ptB[:], start=False, stop=True)
            z1 = work.tile([cfg.HID, P], dt.bfloat16, tag="z1s")
            nc.scalar.activation(z1[:], z1_ps[:], AF.Relu, bias=fb1c[:])

            z2_ps = head_ps.tile([1, P], dt.float32, tag="z2")
            nc.tensor.matmul(out=z2_ps[:], lhsT=fw2[:], rhs=z1[:], start=True, stop=True)
            z2 = work.tile([1, P], dt.float32, tag="z2s")
            nc.scalar.activation(z2[:], z2_ps[:], AF.Copy, bias=float(fb2))
            # softmax over a width-1 axis == 1.0 for finite logits
            outs = work.tile([1, P], dt.float32, tag="outs")
            nc.vector.tensor_tensor(out=outs[:], in0=z2[:], in1=z2[:], op=OP.is_equal)
            nc.sync.dma_start(out=out_d[:], in_=outs[:])

    nc.compile()
    return nc


# ---------------------------------------------------------------- run

_CACHE = {}


def _get_nc(cfg, fb2):
    key = (tuple(cfg.F), cfg.NW, cfg.T, cfg.SHARD, fb2)
    if key not in _CACHE:
        _CACHE[key] = build_bass(cfg, fb2)
    return _CACHE[key]


def make_in_maps(cfg, inp):
    cores, xt, wts, fb2 = build_host_data(cfg, inp)
    in_maps = []
    for c in range(cfg.n_cores):
        m = dict(xt=xt, **cores[c], **wts)
        in_maps.append(m)
    return in_maps, fb2


def kernel(**inputs):
    cfg = CFG
    # auto-size T to the densest destination window of this edge list
    dst = np.asarray(inputs["edge_index"][1]).astype(np.int64).ravel()
    wid = dst // P  # global window id (SHARD multiple of 128); self-loops excluded
    wcnt = np.bincount(wid, minlength=cfg.NPAD // P)
    need_T = max(1, int(-(-wcnt.max() // P)))
    if need_T != cfg.T:
        cfg = GCNConfig(**{**cfg.__dict__, "T": need_T})
    in_maps, fb2 = make_in_maps(cfg, inputs)
    nc = _get_nc(cfg, fb2)
    from concourse.bass_utils import run_bass_kernel_spmd
    res = run_bass_kernel_spmd(nc, in_maps, core_ids=list(range(cfg.n_cores)))
    out = np.asarray(res.results[0]["out"]).reshape(P)[:cfg.G]
    return out.reshape(cfg.G, 1).astype(np.float32)



# revision 3
# speedup vs baseline: 40.1768x; 40.1768x over previous
"""GCN (3-layer GCNConv + mean-pool + MLP head) Trainium2 Bass kernel, 8 NeuronCores.

Strategy (graph/data parallel, per sharding hint):
  - Destination nodes are partitioned into 8 contiguous blocks (one per core).
  - Host partitions the edge list (self-loops included as ordinary edges with
    norm 1/deg) by destination block, sorts by destination window (128 dst
    nodes per window), and lays edges out on a [128, NW*T] grid so each
    128-edge tile feeds one PE matmul.
  - Per layer, each core gathers source-node features for a whole window's
    edges with ONE batched indirect DMA ([128, T] offset AP), multiplies by
    the GCN edge norm, and segment-sums into its destination windows with
    one-hot x message matmuls accumulated in PSUM (aggregate-then-transform:
    A_hat @ (h W) == (A_hat @ h) @ W, so gathers run at the *input* width).
  - The per-shard layer output (relu(agg @ W + b)) is written to a shard
    bounce buffer and AllGathered so every core has the full node-feature
    table for the next layer's gathers.
  - Layer 3 output is mean-pooled per graph locally (one-hot matmul into a
    PSUM accumulator held across the layer), AllReduced across cores, and the
    tiny FC head runs replicated on every core.
"""

import os
import sys
from dataclasses import dataclass, field

import numpy as np
import ml_dtypes

for _p in ("/opt/trn_rl_repo", "/root/.axon_site/_ro/trn_rl_repo"):
    if os.path.isdir(_p) and _p not in sys.path:
        sys.path.insert(0, _p)

bf16 = ml_dtypes.bfloat16
P = 128


@dataclass
class GCNConfig:
    N: int = 100000          # real nodes
    G: int = 128             # graphs (output width; PSUM col budget)
    SHARD: int = 12544       # padded nodes per core (NW * 128)
    NW: int = 98             # dst windows per core
    T: int = 18              # edge columns per window (incl self-loops; auto-derived per input)
    F: tuple = (40, 40, 80, 160)   # feature dims x -> h1 -> h2 -> h3
    HID: int = 128           # fc hidden
    n_cores: int = 8

    @property
    def NPAD(self):
        return self.n_cores * self.SHARD

    @property
    def COLS(self):
        return self.NW * self.T


CFG = GCNConfig()


# ---------------------------------------------------------------- host prep

def build_host_data(cfg, inp):
    """Partition/sort edges (incl self-loops) by destination block, compute GCN
    edge norms, build per-core edge grids and tables."""
    N, SHARD, NW, T = cfg.N, cfg.SHARD, cfg.NW, cfg.T
    src = np.asarray(inp["edge_index"][0]).astype(np.int64).ravel()
    dst = np.asarray(inp["edge_index"][1]).astype(np.int64).ravel()
    batch = np.asarray(inp["batch"]).astype(np.int64).ravel()
    deg = (np.bincount(dst, minlength=N) + 1).astype(np.float32)
    dis = 1.0 / np.sqrt(deg)
    loop = np.arange(N, dtype=np.int64)
    srcA = np.concatenate([src, loop])
    dstA = np.concatenate([dst, loop])
    norm = np.concatenate([(dis[src] * dis[dst]).astype(np.float32),
                           (1.0 / deg).astype(np.float32)])

    core = dstA // SHARD
    win = (dstA % SHARD) // P
    dloc = (dstA % SHARD) % P
    order = np.lexsort((win, core))
    srcA, core, win, dloc, norm = (a[order] for a in (srcA, core, win, dloc, norm))

    cores = []
    for c in range(cfg.n_cores):
        m = core == c
        sc, dl, nm, wn = srcA[m], dloc[m], norm[m], win[m]
        esrc = np.zeros((P, NW * T), np.int32)
        edl = np.full((P, NW * T), -1.0, bf16)
        enrm = np.zeros((P, NW * T), bf16)
        # edges of window w occupy grid slots [p, w*T + j//128]
        wcnt = np.bincount(wn, minlength=NW)
        assert wcnt.max() <= T * P, f"window overflow: {wcnt.max()} > {T * P}"
        jin = np.concatenate([np.arange(n) for n in wcnt]) if len(wn) else np.zeros(0, np.int64)
        pp = jin % P
        cc = wn * T + jin // P
        esrc[pp, cc] = sc
        edl[pp, cc] = dl.astype(bf16)
        enrm[pp, cc] = nm.astype(bf16)
        nid = np.arange(SHARD) + c * SHARD
        gl = np.where(nid < N, batch[np.minimum(nid, N - 1)], -1).astype(np.float32)
        gloc = np.ascontiguousarray(gl.reshape(NW, P).T).astype(bf16)
        cores.append(dict(esrc=esrc, edl=edl, enrm=enrm, gloc=gloc))

    # segment-major table layout: two AllGather segments per layer boundary.
    # node v -> row seg*(n_cores*SEG) + core*SEG + (v%SHARD - seg*SEG),
    # with SEG = SHARD//2; segment s of every core is one collective.
    SEG = SHARD // 2
    vv = np.arange(cfg.NPAD, dtype=np.int64)
    vc, vr = vv // SHARD, vv % SHARD
    vs = vr // SEG
    remap = vs * (cfg.n_cores * SEG) + vc * SEG + (vr - vs * SEG)
    for d in cores:
        d["esrc"] = remap[d["esrc"].astype(np.int64)].astype(np.int32)
    xt = np.zeros((cfg.NPAD, cfg.F[0]), bf16)
    xr = np.asarray(inp["x"]).astype(bf16)
    xt[remap[:N]] = xr

    cnt = np.bincount(batch, minlength=cfg.G).astype(np.float32)
    invc = np.zeros((P, 1), np.float32)
    invc[:cfg.G, 0] = 1.0 / np.maximum(cnt, 1.0)

    def a2(x, dt):
        return np.ascontiguousarray(np.asarray(x), dtype=dt)

    wts = dict(
        w1a=np.concatenate([a2(inp["W1"], bf16), a2(inp["b1"], bf16)[None]], 0),
        w2a=np.concatenate([a2(inp["W2"], bf16), a2(inp["b2"], bf16)[None]], 0),
        w3a=np.concatenate([a2(inp["W3"], bf16), a2(inp["b3"], bf16)[None]], 0),
        fw1=a2(inp["fW1"], bf16),
        fb1c=a2(inp["fb1"], np.float32).reshape(-1, 1),
        fw2=a2(inp["fW2"], bf16),
        invc=invc,
    )
    fb2 = float(np.asarray(inp["fb2"]).ravel()[0])
    return cores, xt, wts, fb2


# ---------------------------------------------------------------- bass build

def build_bass(cfg, fb2):
    import concourse.bacc as bacc
    import concourse.bass as bass
    import concourse.mybir as mybir
    import concourse.tile as tile
    from concourse.masks import make_identity

    dt = mybir.dt
    AF = mybir.ActivationFunctionType
    OP = mybir.AluOpType
    F0, F1, F2, F3 = cfg.F
    NW, T, G = cfg.NW, cfg.T, cfg.G
    FMAX = max(F0, F1, F2)

    nc = bacc.Bacc("TRN2", target_bir_lowering=False, debug=False,
                   enable_asserts=False, num_devices=cfg.n_cores,
                   num_swdge_queues=4)

    # ---- I/O
    xt_d = nc.dram_tensor("xt", [cfg.NPAD, F0], dt.bfloat16, kind="ExternalInput")
    esrc_d = nc.dram_tensor("esrc", [P, cfg.COLS], dt.int32, kind="ExternalInput")
    edl_d = nc.dram_tensor("edl", [P, cfg.COLS], dt.bfloat16, kind="ExternalInput")
    enrm_d = nc.dram_tensor("enrm", [P, cfg.COLS], dt.bfloat16, kind="ExternalInput")
    gloc_d = nc.dram_tensor("gloc", [P, NW], dt.bfloat16, kind="ExternalInput")
    w1a_d = nc.dram_tensor("w1a", [F0 + 1, F1], dt.bfloat16, kind="ExternalInput")
    w2a_d = nc.dram_tensor("w2a", [F1 + 1, F2], dt.bfloat16, kind="ExternalInput")
    w3a_d = nc.dram_tensor("w3a", [F2 + 1, F3], dt.bfloat16, kind="ExternalInput")
    fw1_d = nc.dram_tensor("fw1", [F3, cfg.HID], dt.bfloat16, kind="ExternalInput")
    fb1_d = nc.dram_tensor("fb1c", [cfg.HID, 1], dt.float32, kind="ExternalInput")
    fw2_d = nc.dram_tensor("fw2", [cfg.HID, 1], dt.bfloat16, kind="ExternalInput")
    invc_d = nc.dram_tensor("invc", [P, 1], dt.float32, kind="ExternalInput")
    out_d = nc.dram_tensor("out", [1, P], dt.float32, kind="ExternalOutput")

    rg = [list(range(cfg.n_cores))]

    with tile.TileContext(nc) as tc:
        with (
            tc.tile_pool(name="res", bufs=1) as res,                  # persistent SBUF
            tc.tile_pool(name="msgs", bufs=3) as msgsp,
            tc.tile_pool(name="sp", bufs=3) as sp,
            tc.tile_pool(name="work", bufs=2) as work,
            tc.tile_pool(name="pa_ps", bufs=2, space="PSUM") as pa_ps,
            tc.tile_pool(name="p2_ps", bufs=2, space="PSUM") as p2_ps,
            tc.tile_pool(name="pool_ps", bufs=1, space="PSUM") as pool_ps,
            tc.tile_pool(name="head_ps", bufs=1, space="PSUM") as head_ps,
            tc.tile_pool(name="dram", bufs=1, space="DRAM") as dram,
        ):
            # ---- load persistent SBUF state
            esrc = res.tile([P, cfg.COLS], dt.int32)
            edl = res.tile([P, cfg.COLS], dt.bfloat16)
            enrm = res.tile([P, cfg.COLS], dt.bfloat16)
            gloc = res.tile([P, NW], dt.bfloat16)
            w1a = res.tile([F0 + 1, F1], dt.bfloat16)
            w2a = res.tile([F1 + 1, F2], dt.bfloat16)
            w3a = res.tile([F2 + 1, F3], dt.bfloat16)
            fw1a = res.tile([F3 // 2, cfg.HID], dt.bfloat16)
            fw1b = res.tile([F3 // 2, cfg.HID], dt.bfloat16)
            fb1c = res.tile([cfg.HID, 1], dt.float32)
            fw2 = res.tile([cfg.HID, 1], dt.bfloat16)
            invc = res.tile([P, 1], dt.float32)
            b1r = res.tile([1, F1], dt.bfloat16)
            b2r = res.tile([1, F2], dt.bfloat16)
            b3r = res.tile([1, F3], dt.bfloat16)
            for sb, dr in ((esrc, esrc_d), (edl, edl_d), (enrm, enrm_d),
                           (gloc, gloc_d), (w1a, w1a_d), (w2a, w2a_d),
                           (w3a, w3a_d), (fb1c, fb1_d), (fw2, fw2_d),
                           (invc, invc_d)):
                nc.sync.dma_start(out=sb[:], in_=dr[:])
            nc.sync.dma_start(out=b1r[:], in_=w1a_d[F0:F0 + 1, :])
            nc.sync.dma_start(out=b2r[:], in_=w2a_d[F1:F1 + 1, :])
            nc.sync.dma_start(out=b3r[:], in_=w3a_d[F2:F2 + 1, :])
            nc.sync.dma_start(out=fw1a[:], in_=fw1_d[0:F3 // 2, :])
            nc.sync.dma_start(out=fw1b[:], in_=fw1_d[F3 // 2:, :])

            iota_i = res.tile([P, P], dt.int32)
            nc.gpsimd.iota(iota_i[:], pattern=[[1, P]], base=0, channel_multiplier=0)
            iota_b = res.tile([P, P], dt.bfloat16)
            nc.vector.tensor_copy(out=iota_b[:], in_=iota_i[:])
            ident = res.tile([P, P], dt.bfloat16)
            make_identity(nc, ident[:])
            ones1 = res.tile([1, P], dt.bfloat16)
            nc.vector.memset(ones1[:], 1.0)

            # ---- DRAM tables / bounce buffers
            h1s = dram.tile([cfg.SHARD, F1], dt.bfloat16)
            h2s = dram.tile([cfg.SHARD, F2], dt.bfloat16)
            h1t = dram.tile([cfg.NPAD, F1], dt.bfloat16)
            h2t = dram.tile([cfg.NPAD, F2], dt.bfloat16)
            pool_pt = dram.tile([P, F3], dt.float32)
            pool_rd = dram.tile([P, F3], dt.float32)

            pool_acc = pool_ps.tile([P, F3], dt.float32)

            def layer(tbl, F_in, F_out, waug, brow, shard_out):
                last = F_in == F2  # layer 3
                for w in range(NW):
                    wc = slice(w * T, (w + 1) * T)
                    msgs = msgsp.tile([P, T, FMAX], dt.bfloat16, tag="msgs", name="msgs")
                    msgs = msgs[:, :, :F_in]
                    gi = nc.gpsimd.indirect_dma_start(
                        out=msgs[:],
                        out_offset=None,
                        in_=tbl[:],
                        in_offset=bass.IndirectOffsetOnAxis(ap=esrc[:, wc], axis=0),
                    )
                    q = w % 4
                    if q:
                        gi.queue = f"qPoolDynamic{q}"
                    S = sp.tile([P, T, P], dt.bfloat16, tag="S", name="S")
                    nc.vector.tensor_tensor(
                        out=S[:],
                        in0=edl[:, wc, None].broadcast_to([P, T, P]),
                        in1=iota_b[:, None, :].broadcast_to([P, T, P]),
                        op=OP.is_equal)
                    nc.vector.tensor_tensor(
                        out=msgs, in0=msgs,
                        in1=enrm[:, wc, None].broadcast_to([P, T, F_in]),
                        op=OP.mult)
                    pa = pa_ps.tile([FMAX, P], dt.float32, tag="pa", name="pa")[:F_in]
                    for t in range(T):
                        nc.tensor.matmul(
                            out=pa[:], lhsT=msgs[:, t, :], rhs=S[:, t, :],
                            start=(t == 0), stop=(t == T - 1))
                    aggT = work.tile([FMAX, P], dt.bfloat16, tag="aggT", name="aggT")[:F_in]
                    nc.scalar.copy(out=aggT[:], in_=pa[:])
                    p2 = p2_ps.tile([P, F3], dt.float32, tag="p2", name="p2")[:, :F_out]
                    nc.tensor.matmul(out=p2[:], lhsT=aggT[:], rhs=waug[:F_in, :],
                                     start=True, stop=False)
                    nc.tensor.matmul(out=p2[:], lhsT=ones1[:], rhs=brow[:],
                                     start=False, stop=True)
                    h = work.tile([P, F3], dt.bfloat16, tag="h", name="h")[:, :F_out]
                    nc.scalar.activation(h[:], p2[:], AF.Relu)
                    if not last:
                        nc.sync.dma_start(out=shard_out[w * P:(w + 1) * P, :], in_=h[:])
                    else:
                        Sg = sp.tile([P, P], dt.bfloat16, tag="Sg", name="Sg")
                        nc.vector.tensor_tensor(
                            out=Sg[:],
                            in0=gloc[:, w:w + 1].broadcast_to([P, P]),
                            in1=iota_b[:], op=OP.is_equal)
                        nc.tensor.matmul(out=pool_acc[:], lhsT=Sg[:], rhs=h[:],
                                         start=(w == 0), stop=(w == NW - 1))

            SEG = cfg.SHARD // 2
            HSEG = cfg.n_cores * SEG

            def seg_allgather(shard, table):
                for sgi in range(2):
                    nc.gpsimd.collective_compute(
                        "AllGather", mybir.AluOpType.bypass, replica_groups=rg,
                        ins=[shard[sgi * SEG:(sgi + 1) * SEG, :].opt()],
                        outs=[table[sgi * HSEG:(sgi + 1) * HSEG, :].opt()])

            layer(xt_d, F0, F1, w1a, b1r, h1s)
            seg_allgather(h1s, h1t)
            layer(h1t, F1, F2, w2a, b2r, h2s)
            seg_allgather(h2s, h2t)
            layer(h2t, F2, F3, w3a, b3r, None)

            # ---- pooling partial -> AllReduce -> mean
            psb = work.tile([P, F3], dt.float32, tag="psb")
            nc.scalar.copy(out=psb[:], in_=pool_acc[:])
            nc.sync.dma_start(out=pool_pt[:], in_=psb[:])
            nc.gpsimd.collective_compute(
                "AllReduce", mybir.AluOpType.add, replica_groups=rg,
                ins=[pool_pt.opt()], outs=[pool_rd.opt()])
            poolr = work.tile([P, F3], dt.float32, tag="poolr")
            nc.sync.dma_start(out=poolr[:], in_=pool_rd[:])
            pooled = work.tile([P, F3], dt.bfloat16, tag="pooled")
            nc.scalar.activation(pooled[:], poolr[:], AF.Copy, scale=invc[:])

            # ---- head: z1 = relu(pooled @ fW1 + fb1); z2 = z1 @ fW2 + fb2
            ptA_ps = head_ps.tile([F3 // 2, P], dt.bfloat16, tag="pt")
            nc.tensor.transpose(out=ptA_ps[:], in_=pooled[:, :F3 // 2], identity=ident[:])
            ptA = work.tile([F3 // 2, P], dt.bfloat16, tag="ptA")
            nc.scalar.copy(out=ptA[:], in_=ptA_ps[:])
            ptB_ps = head_ps.tile([F3 // 2, P], dt.bfloat16, tag="pt")
            nc.tensor.transpose(out=ptB_ps[:], in_=pooled[:, F3 // 2:], identity=ident[:])
            ptB = work.tile([F3 // 2, P], dt.bfloat16, tag="ptB")
            nc.scalar.copy(out=ptB[:], in_=ptB_ps[:])

            z1_ps = head_ps.tile([cfg.HID, P], dt.float32, tag="z1")
            nc.tensor.matmul(out=z1_ps[:], lhsT=fw1a[:], rhs=ptA[:], start=True, stop=False)
            nc.tensor.matmul(out=z1_ps[:], lhsT=fw1b[:], rhs=ptB[:], start=False, stop=True)
            z1 = work.tile([cfg.HID, P], dt.bfloat16, tag="z1s")
            nc.scalar.activation(z1[:], z1_ps[:], AF.Relu, bias=fb1c[:])

            z2_ps = head_ps.tile([1, P], dt.float32, tag="z2")
            nc.tensor.matmul(out=z2_ps[:], lhsT=fw2[:], rhs=z1[:], start=True, stop=True)
            z2 = work.tile([1, P], dt.float32, tag="z2s")
            nc.scalar.activation(z2[:], z2_ps[:], AF.Copy, bias=float(fb2))
            # softmax over a width-1 axis == 1.0 for finite logits
            outs = work.tile([1, P], dt.float32, tag="outs")
            nc.vector.tensor_tensor(out=outs[:], in0=z2[:], in1=z2[:], op=OP.is_equal)
            nc.sync.dma_start(out=out_d[:], in_=outs[:])

    nc.compile()
    return nc


# ---------------------------------------------------------------- run

_CACHE = {}


def _get_nc(cfg, fb2):
    key = (tuple(cfg.F), cfg.NW, cfg.T, cfg.SHARD, fb2)
    if key not in _CACHE:
        _CACHE[key] = build_bass(cfg, fb2)
    return _CACHE[key]


def make_in_maps(cfg, inp):
    cores, xt, wts, fb2 = build_host_data(cfg, inp)
    in_maps = []
    for c in range(cfg.n_cores):
        m = dict(xt=xt, **cores[c], **wts)
        in_maps.append(m)
    return in_maps, fb2


def derive_cfg(inputs):
    """Auto-size T to the densest destination window (self-loops included)."""
    cfg = CFG
    dst = np.asarray(inputs["edge_index"][1]).astype(np.int64).ravel()
    wid = np.concatenate([dst, np.arange(cfg.N, dtype=np.int64)]) // P
    wcnt = np.bincount(wid, minlength=cfg.NPAD // P)
    need_T = max(1, int(-(-wcnt.max() // P)))
    if need_T != cfg.T:
        cfg = GCNConfig(**{**cfg.__dict__, "T": need_T})
    return cfg


def kernel(**inputs):
    cfg = derive_cfg(inputs)
    in_maps, fb2 = make_in_maps(cfg, inputs)
    nc = _get_nc(cfg, fb2)
    from concourse.bass_utils import run_bass_kernel_spmd
    res = run_bass_kernel_spmd(nc, in_maps, core_ids=list(range(cfg.n_cores)))
    out = np.asarray(res.results[0]["out"]).reshape(P)[:cfg.G]
    return out.reshape(cfg.G, 1).astype(np.float32)
